# revision 9
# baseline (speedup 1.0000x reference)
"""Multi-head graph attention message passing on 8 Trainium2 cores.

Design (v3, dst-stationary, group-batched):
  - Nodes sharded by dst across 8 cores (12500 each).
  - Per core and per src-chunk (4 chunks of 25600 nodes, so gather indices
    fit int16), the core's dst nodes are sorted by their degree *within that
    chunk* and grouped into blocks of 128 (one dst node per SBUF partition).
  - Blocks are packed into gather groups with a uniform slot width Wg
    (= width of the group's first block; widths are descending, so padding
    is small). One dma_gather per group fetches KV rows (bf16, 512B) for all
    edges; slot (p, k*Wg+s) holds the s-th edge of block k's p-th dst node.
    Padding slots point at a zeroed table row (V=0 => contributes nothing).
  - Q is never gathered: the host pre-permutes x into per-chunk dst order
    and Q is projected per block straight into SBUF (matmul + bias).
  - All edge math runs as a handful of large per-group DVE ops (bf16), incl.
    the slot reduction (reduce over an inner-strided axis); per-chunk partial
    wV is written contiguously. The host sums the 4 permuted partials.
  - SWDGE descriptor generation (Q7) is the bottleneck (~4-5 ns/descriptor);
    gathers rotate over 4 SWDGE queues (~1.8x concurrency) with large calls
    (single_packet=False lifts the 64-descriptor packet cap).
"""

import numpy as np
import ml_dtypes

import concourse.bacc as bacc
import concourse.mybir as mybir
import concourse.tile as tile
from concourse.bass_utils import run_bass_kernel_spmd

F32 = mybir.dt.float32
BF16 = mybir.dt.bfloat16
I16 = mybir.dt.int16


class Cfg:
    n_nodes = 100000
    n_edges = 1600000
    in_dim = 128
    heads = 8
    hdim = 16
    hid = 128            # heads * hdim
    n_cores = 8
    n_chunks = 4         # src chunks so int16 gather indices stay in range
    proj_tile = 512      # nodes per projection matmul group
    group_slots = 48     # max slots (nb*Wg) per gather call; x128 = num_idxs
    group_blocks = 12    # max blocks per group
    n_queues = 4         # SWDGE queues to rotate gathers over
    dma_scratch = 16384  # descriptor-ring carveout bytes per partition
    clip_margin = 19.5   # |score| beyond this forces on-device clamping
    reduce_mode = "strided"  # "strided" | "chain" slot reduction
    x_bf16 = True        # upload x / weights in bf16 (halves projection reads)

    def __init__(self, **kw):
        for k, v in kw.items():
            setattr(self, k, v)
        assert self.n_nodes % self.n_cores == 0
        self.own = self.n_nodes // self.n_cores          # 12500
        self.own_pad = -(-self.own // 128) * 128         # 12544
        self.n_blocks = self.own_pad // 128              # 98
        self.cr = 25600                                  # chunk rows
        assert self.cr * self.n_chunks >= self.n_nodes
        self.tab_rows = self.cr + 128                    # + zero pad rows
        self.np_pad = self.cr * self.n_chunks            # 102400
        self.dummy_row = self.cr                         # chunk-local zero row
        self.xdt = BF16 if self.x_bf16 else F32
        self.xnp = ml_dtypes.bfloat16 if self.x_bf16 else np.float32


def build_program(cfg, plan):
    """One SPMD program; per-core behavior differs only through input data."""
    nc = bacc.Bacc("TRN2", target_bir_lowering=False, debug=False,
                   num_swdge_queues=cfg.n_queues,
                   dynamic_dma_scratch_size=cfg.dma_scratch)

    XDT = cfg.xdt
    xt = nc.dram_tensor("xt", [cfg.in_dim, cfg.np_pad], XDT, kind="ExternalInput")
    xq = nc.dram_tensor("xq", [cfg.n_chunks, cfg.in_dim, cfg.own_pad], XDT,
                        kind="ExternalInput")
    w_kv = nc.dram_tensor("w_kv", [cfg.in_dim, 2 * cfg.hid], XDT, kind="ExternalInput")
    w_q = nc.dram_tensor("w_q", [cfg.in_dim, cfg.hid], XDT, kind="ExternalInput")
    b_kv = nc.dram_tensor("b_kv", [128, 2 * cfg.hid], F32, kind="ExternalInput")
    b_q4 = nc.dram_tensor("b_q4", [128, 4 * cfg.hid], F32, kind="ExternalInput")
    eidx = nc.dram_tensor("eidx", [128, plan["total_cols"]], I16,
                          kind="ExternalInput")
    wv = nc.dram_tensor("wv", [cfg.n_chunks, cfg.own_pad, cfg.hid], F32,
                        kind="ExternalOutput")

    kv_tabs = [nc.dram_tensor(f"kv_tab{c}", [cfg.tab_rows, 2 * cfg.hid], BF16)
               for c in range(cfg.n_chunks)]

    PT = cfg.proj_tile
    scale = float(np.sqrt(cfg.hdim))
    lim = 5.0 * scale

    with tile.TileContext(nc) as tc:
        with (
            tc.tile_pool(name="const", bufs=1) as cpool,
            tc.tile_pool(name="proj", bufs=3) as ppool,
            tc.tile_pool(name="psum", bufs=4, space="PSUM") as psum,
            tc.tile_pool(name="qsum", bufs=4, space="PSUM") as qsum,
            tc.tile_pool(name="edge", bufs=2) as epool,
            tc.tile_pool(name="mid", bufs=2) as mpool,
            tc.tile_pool(name="idx", bufs=3) as ipool,
            tc.tile_pool(name="blk", bufs=3) as bpool,
            tc.tile_pool(name="out", bufs=3) as opool,
        ):
            w_kv_t = cpool.tile([cfg.in_dim, 2 * cfg.hid], XDT)
            w_q_t = cpool.tile([cfg.in_dim, cfg.hid], XDT)
            b_kv_t = cpool.tile([128, 2 * cfg.hid], F32)
            b_q4_t = cpool.tile([128, 4 * cfg.hid], F32)
            nc.sync.dma_start(w_kv_t[:], w_kv[:])
            nc.sync.dma_start(w_q_t[:], w_q[:])
            nc.sync.dma_start(b_kv_t[:], b_kv[:])
            nc.sync.dma_start(b_q4_t[:], b_q4[:])

            # zero pad rows of each chunk table (gather target for padding)
            zt = cpool.tile([128, 2 * cfg.hid], BF16)
            nc.vector.memset(zt[:], 0.0)
            for ch in range(cfg.n_chunks):
                nc.sync.dma_start(kv_tabs[ch][cfg.cr:cfg.tab_rows, :], zt[:])

            # K|V projection, chunk by chunk (gathers of chunk ch wait only
            # on chunk ch's table writes). PSUM batches 2 node-tiles per bank.
            for ch in range(cfg.n_chunks):
                for g in range(cfg.cr // PT):
                    c0 = ch * cfg.cr + g * PT
                    xt_t = ppool.tile([128, PT], XDT, tag="xt_t")
                    nc.sync.dma_start(xt_t[:], xt[:, c0:c0 + PT])
                    out_sb = ppool.tile([128, PT // 128, 2 * cfg.hid], BF16,
                                        tag="out_sb")
                    for s in range(0, PT // 128, 2):
                        ps = psum.tile([128, 2, 2 * cfg.hid], F32)
                        for j in range(2):
                            nc.tensor.matmul(
                                ps[:, j, :],
                                xt_t[:, (s + j) * 128:(s + j + 1) * 128],
                                w_kv_t[:], start=True, stop=True)
                        nc.vector.tensor_add(
                            out_sb[:, s:s + 2, :], ps[:],
                            b_kv_t[:].unsqueeze(1).broadcast_to(
                                [128, 2, 2 * cfg.hid]))
                    dview = kv_tabs[ch][g * PT:(g + 1) * PT, :].rearrange(
                        "(s p) e -> p s e", p=128)
                    nc.sync.dma_start(dview, out_sb[:])

            # edge phase: one gather + a handful of large DVE ops per group
            for gi, (ch, b0, nb, Wg, col0) in enumerate(plan["groups"]):
                S = nb * Wg
                n_idx = S * 128
                it = ipool.tile([128, n_idx // 16], I16, tag="it")
                nc.sync.dma_start(it[:], eidx[:, col0:col0 + n_idx // 16])
                gt = epool.tile([128, S, 2 * cfg.hid], BF16, tag="gt")
                nc.gpsimd.dma_gather(
                    gt[:], kv_tabs[ch][:], it[:], n_idx, n_idx, 2 * cfg.hid,
                    queue_num=gi % cfg.n_queues,
                    single_packet=n_idx <= 1024)

                xq_t = bpool.tile([128, nb * 128], XDT, tag="xq_t")
                nc.sync.dma_start(
                    xq_t[:], xq[ch, :, b0 * 128:(b0 + nb) * 128])
                qg = bpool.tile([128, nb, cfg.hid], BF16, tag="qg")
                for j0 in range(0, nb, 4):
                    cc = min(4, nb - j0)
                    qp = qsum.tile([128, cc, cfg.hid], F32)
                    for j in range(cc):
                        nc.tensor.matmul(
                            qp[:, j, :],
                            xq_t[:, (j0 + j) * 128:(j0 + j + 1) * 128],
                            w_q_t[:], start=True, stop=True)
                    nc.vector.tensor_add(
                        qg[:, j0:j0 + cc, :], qp[:],
                        b_q4_t[:, :cc * cfg.hid].rearrange(
                            "p (c f) -> p c f", f=cfg.hid))

                gv = gt[:].rearrange("p (n w) e -> p n w e", w=Wg)
                prod = mpool.tile([128, S, cfg.hid], BF16, tag="prod")
                nc.vector.tensor_mul(
                    prod[:].rearrange("p (n w) f -> p n w f", w=Wg),
                    gv[:, :, :, :cfg.hid],
                    qg[:].unsqueeze(2).broadcast_to([128, nb, Wg, cfg.hid]))
                sc = bpool.tile([128, S, cfg.heads], F32, tag="sc")
                nc.vector.reduce_sum(
                    sc[:],
                    prod[:].rearrange("p s (h d) -> p s h d", d=cfg.hdim),
                    axis=mybir.AxisListType.X)
                if plan["need_clip"]:
                    nc.vector.tensor_scalar_min(sc[:], sc[:], lim)
                    nc.vector.tensor_scalar_max(sc[:], sc[:], -lim)
                ex = bpool.tile([128, S, cfg.heads], BF16, tag="ex")
                nc.scalar.activation(
                    ex[:], sc[:], mybir.ActivationFunctionType.Exp,
                    scale=float(1.0 / scale))

                msg = mpool.tile([128, S, cfg.hid], BF16, tag="msg")
                nc.vector.tensor_mul(
                    msg[:].rearrange("p s (h d) -> p s h d", d=cfg.hdim),
                    gt[:, :, cfg.hid:].rearrange(
                        "p s (h d) -> p s h d", d=cfg.hdim),
                    ex[:].unsqueeze(-1).broadcast_to(
                        [128, S, cfg.heads, cfg.hdim]))

                ov = opool.tile([128, nb, cfg.hid], F32, tag="ov")
                if cfg.reduce_mode == "strided" and Wg > 1:
                    nc.vector.reduce_sum(
                        ov[:],
                        msg[:].rearrange("p (n w) f -> p n f w", w=Wg),
                        axis=mybir.AxisListType.X)
                else:
                    mv = msg[:].rearrange("p (n w) f -> p n w f", w=Wg)
                    if Wg == 1:
                        nc.vector.tensor_scalar_add(ov[:], mv[:, :, 0, :], 0.0)
                    else:
                        nc.vector.tensor_add(ov[:], mv[:, :, 0, :],
                                             mv[:, :, 1, :])
                        for s in range(2, Wg):
                            nc.vector.tensor_add(ov[:], ov[:], mv[:, :, s, :])

                dview = wv[ch, b0 * 128:(b0 + nb) * 128, :].rearrange(
                    "(s p) e -> p s e", p=128)
                nc.sync.dma_start(dview, ov[:])
    nc.finalize()
    return nc


def _wrap16(a):
    """[n] -> [128, n//16] int16: idx i at [i%16 (+16k replicas), i//16]."""
    w = a.reshape(-1, 16).T.astype(np.int16)
    return np.tile(w, (8, 1))


def prepare_inputs(cfg, x, src, dst, Wq, bq, Wk, bk, Wv, bv):
    x = np.asarray(x, np.float32)
    src = np.asarray(src, np.int64)
    dst = np.asarray(dst, np.int64)

    xt = np.zeros((cfg.in_dim, cfg.np_pad), cfg.xnp)
    xt[:, :cfg.n_nodes] = x.T.astype(cfg.xnp)
    w_kv = np.concatenate([np.asarray(Wk, np.float32),
                           np.asarray(Wv, np.float32)], axis=1).astype(cfg.xnp)
    b_kv = np.tile(np.concatenate([np.asarray(bk, np.float32),
                                   np.asarray(bv, np.float32)])[None, :], (128, 1))
    w_q = np.asarray(Wq, np.float32).astype(cfg.xnp)
    b_q4 = np.tile(np.asarray(bq, np.float32)[None, :], (128, 4))

    core_of = dst // cfg.own
    chunk_of = src // cfg.cr

    # per (core, chunk): degree-sort dst nodes, assign edge slots
    perms = np.empty((cfg.n_cores, cfg.n_chunks, cfg.own_pad), np.int64)
    ew = {}   # (c, ch) -> (rank, within-rank position, chunk-local src)
    wcc = np.zeros((cfg.n_cores, cfg.n_chunks, cfg.n_blocks), np.int64)
    for c in range(cfg.n_cores):
        in_c = np.nonzero(core_of == c)[0]
        ch_all = chunk_of[in_c]
        for ch in range(cfg.n_chunks):
            e = in_c[ch_all == ch]
            dl = dst[e] - c * cfg.own
            sl = src[e] - ch * cfg.cr
            cnt = np.bincount(dl, minlength=cfg.own_pad)
            perm = np.argsort(-cnt, kind="stable")
            perms[c, ch] = perm
            scnt = cnt[perm]
            wcc[c, ch] = scnt[::128][:cfg.n_blocks]
            rank = np.empty(cfg.own_pad, np.int64)
            rank[perm] = np.arange(cfg.own_pad)
            r = rank[dl]
            o = np.argsort(r, kind="stable")
            rs, sls = r[o], sl[o]
            starts = np.cumsum(scnt) - scnt
            pos = np.arange(len(e)) - starts[rs]
            ew[(c, ch)] = (rs, pos, sls)

    W = np.maximum(wcc.max(axis=0), 1)  # [n_chunks, n_blocks], shared program

    # pack blocks into gather groups with uniform width (widths descending)
    groups = []
    col = 0
    gbase = np.zeros((cfg.n_chunks, cfg.n_blocks), np.int64)  # slot base
    gwidth = np.zeros((cfg.n_chunks, cfg.n_blocks), np.int64)
    for ch in range(cfg.n_chunks):
        b = 0
        while b < cfg.n_blocks:
            Wg = int(W[ch, b])
            nb = 1
            while (b + nb < cfg.n_blocks and (nb + 1) * Wg <= cfg.group_slots
                   and nb < cfg.group_blocks):
                nb += 1
            for k in range(nb):
                gbase[ch, b + k] = col * 16 // 128 + k * Wg
                gwidth[ch, b + k] = Wg
            groups.append((ch, b, nb, Wg, col))
            col += nb * Wg * 128 // 16
            b += nb
    total_cols = col

    # per-core edge index arrays
    in_maps = []
    for c in range(cfg.n_cores):
        flat = np.full(total_cols * 16, cfg.dummy_row, np.int64)
        for ch in range(cfg.n_chunks):
            rs, pos, sls = ew[(c, ch)]
            b = rs // 128
            p = rs % 128
            fp = (gbase[ch, b] + pos) * 128 + p
            flat[fp] = sls
        eidx = _wrap16(flat)

        xqa = np.zeros((cfg.n_chunks, cfg.in_dim, cfg.own_pad), cfg.xnp)
        x_loc = x[c * cfg.own:(c + 1) * cfg.own]
        for ch in range(cfg.n_chunks):
            pm = perms[c, ch]
            xp = x_loc[np.clip(pm, 0, cfg.own - 1)]
            xp[pm >= cfg.own] = 0.0
            xqa[ch] = xp.T.astype(cfg.xnp)
        in_maps.append({
            "xt": xt, "xq": xqa,
            "w_kv": w_kv, "w_q": w_q, "b_kv": b_kv, "b_q4": b_q4,
            "eidx": eidx,
        })

    # does the data ever reach the clip boundary?
    K = (x @ np.asarray(Wk, np.float32) + np.asarray(bk, np.float32))
    Q = (x @ np.asarray(Wq, np.float32) + np.asarray(bq, np.float32))
    Kh = K.reshape(cfg.n_nodes, cfg.heads, cfg.hdim)
    Qh = Q.reshape(cfg.n_nodes, cfg.heads, cfg.hdim)
    mx = 0.0
    for a in range(0, cfg.n_edges, 200000):
        b_ = min(a + 200000, cfg.n_edges)
        d = np.einsum("ehd,ehd->eh", Kh[src[a:b_]], Qh[dst[a:b_]])
        mx = max(mx, float(np.abs(d).max()))
    need_clip = mx >= cfg.clip_margin

    plan = {"groups": groups, "total_cols": total_cols,
            "need_clip": need_clip, "max_score": mx}
    return in_maps, plan, perms


def postprocess(cfg, results, perms):
    outs = []
    for c in range(cfg.n_cores):
        wv = results[c]["wv"]  # [n_chunks, own_pad, hid]
        acc = np.zeros((cfg.own_pad, cfg.hid), np.float32)
        for ch in range(cfg.n_chunks):
            acc[perms[c, ch]] += wv[ch]
        outs.append(acc[:cfg.own])
    out = np.concatenate(outs, axis=0)
    return out.reshape(cfg.n_nodes, cfg.heads, cfg.hdim)


def kernel(x, src, dst, Wq, bq, Wk, bk, Wv, bv):
    cfg = Cfg()
    in_maps, plan, perms = prepare_inputs(
        cfg, x, src, dst, Wq, bq, Wk, bk, Wv, bv)
    nc = build_program(cfg, plan)
    res = run_bass_kernel_spmd(nc, in_maps, list(range(cfg.n_cores)))
    return postprocess(cfg, res.results, perms)
